# revision 29
# baseline (speedup 1.0000x reference)
"""Expert-parallel MoE (top-2, SwiGLU experts) for 8 Trainium2 NeuronCores.

Strategy: the router (softmax over E=8 experts on [T=4096, H=1024] tokens)
is tiny and runs on host in float64; top-2 selection is stable because the
rank-2/rank-3 logit gap is >>fp32 noise.  Tokens are dispatched by expert id
(the "all-to-all" of the sharding hint happens host-side during sharding):
core e receives the tokens routed to expert e (padded to a fixed capacity)
plus expert e's weights, pre-transposed/pre-tiled into PE-friendly layouts.
Each core computes  interT = silu(wg @ xT) * (wu @ xT);  yT = wd @ interT
with fp32r (TF32-like, full-rate) matmuls accumulating in fp32 PSUM.  The
host applies the top-2 combine weights and scatter-adds into the output.
"""

import sys

for _p in ("/opt/trn_rl_repo", "/root/.axon_site/_ro/trn_rl_repo"):
    if _p not in sys.path:
        sys.path.append(_p)

import numpy as np

B, S, H, I, E = 4, 1024, 1024, 2048, 8
T = B * S
TOP_K = 2
AUX_COEF = 0.01
Z_COEF = 0.001
N_CORES = 8

# Per-expert token capacity.  Mean count is 1024 (T*K/E); the few tokens of
# over-full experts (104 token-pairs for the fixed dataset, max count 1071)
# are computed exactly on host.  1024 = two clean 512-wide matmul chunks --
# fewer rows and 33% fewer PE instructions than covering the max count.
C_PAD = 1024
# matmul moving-dim chunks; each >=256 keeps fp32r at 1 cycle/row, each even
CHUNKS = [(0, 512), (512, 512)]
HC = H // 128  # 8 contraction chunks of 128
IT = I // 128  # 16 intermediate tiles of 128
HT = H // 128  # 8 output tiles of 128

_compiled_nc = None


def _build_program(use_silu=True):
    import concourse.bacc as bacc
    import concourse.tile as tile
    from concourse import mybir

    F32 = mybir.dt.float32
    F32R = mybir.dt.float32r

    nc = bacc.Bacc("TRN2", target_bir_lowering=False, debug=False,
                   num_devices=N_CORES)

    # [p, hc, c] with h = hc*128 + p
    x_d = nc.dram_tensor("xT", [128, HC, C_PAD], F32R, kind="ExternalInput")
    # [it, p, hc, i] = w_gate[e, it*128+i, hc*128+p]
    wg_d = nc.dram_tensor("wg", [IT, 128, HC, 128], F32R, kind="ExternalInput")
    wu_d = nc.dram_tensor("wu", [IT, 128, HC, 128], F32R, kind="ExternalInput")
    # [ht, p, ic, h] = w_down[e, ht*128+h, ic*128+p]
    wd_d = nc.dram_tensor("wd", [HT, 128, IT, 128], F32R, kind="ExternalInput")
    # [ht, p, c] with out-h = ht*128 + p
    y_d = nc.dram_tensor("yT", [HT, 128, C_PAD], F32, kind="ExternalOutput")

    with tile.TileContext(nc) as tc:
        with (
            tc.tile_pool(name="xpool", bufs=1) as xpool,
            tc.tile_pool(name="ipool", bufs=1) as ipool,
            tc.tile_pool(name="wpool", bufs=3) as wpool,
            tc.tile_pool(name="spool", bufs=3) as spool,
            tc.tile_pool(name="ypool", bufs=2) as ypool,
            tc.tile_pool(name="psA", bufs=2, space="PSUM") as psA,
            tc.tile_pool(name="psB", bufs=2, space="PSUM") as psB,
            tc.tile_pool(name="warm", bufs=1, space="PSUM") as warm_pool,
        ):
            x_sb = xpool.tile([128, HC, C_PAD], F32R)
            # Load x by column-chunk, interleaved with the first weight
            # tiles, so the first matmul group starts after ~2MB instead of
            # after the whole 4.4MB+weights (the DMA ring drains in FIFO
            # order; modeled trace showed a 16.5us PE startup gap otherwise).
            c0_first, cs_first = CHUNKS[0]
            # DMA order for fastest useful stream drain: wg[0] -> first x
            # column-chunk -> wu[0] -> remaining chunks/weights (FIFO ring
            # drains in issue order; loading all of x first left a 16.5us
            # PE startup gap).  The first ~16us are DMA-stream-bound, so
            # only stream composition matters, not PE start time.
            wg0_sb = wpool.tile([128, HC, 128], F32R, tag="wg")
            nc.sync.dma_start(wg0_sb[:], wg_d[0])
            nc.sync.dma_start(x_sb[:, :, c0_first:c0_first + cs_first],
                              x_d[:, :, c0_first:c0_first + cs_first])
            wu0_sb = wpool.tile([128, HC, 128], F32R, tag="wu")
            nc.sync.dma_start(wu0_sb[:], wu_d[0])
            # remaining x chunks stream while it=0 computes on chunk 0
            for c0n, csn in CHUNKS[1:]:
                nc.sync.dma_start(x_sb[:, :, c0n:c0n + csn],
                                  x_d[:, :, c0n:c0n + csn])
            inter_sb = ipool.tile([128, IT, C_PAD], F32R)

            # Warm-up: dummy matmuls on the first weight tile while the x
            # DMAs drain, so the PE clock (HAM p-state) reaches full rate
            # before the first real matmul.  Results go to a scratch PSUM
            # bank nobody reads.
            warm_ps = warm_pool.tile([128, 128], F32)
            for _ in range(19):
                nc.tensor.matmul(warm_ps[:], wg0_sb[:, 0, :],
                                 wg0_sb[:, 0, :], start=True, stop=True)

            # Stage A: interT[i, c] = silu(gateT) * upT, tiled over i
            for it in range(IT):
                if it == 0:
                    wg_sb, wu_sb = wg0_sb, wu0_sb
                else:
                    wg_sb = wpool.tile([128, HC, 128], F32R, tag="wg")
                    nc.sync.dma_start(wg_sb[:], wg_d[it])
                    wu_sb = wpool.tile([128, HC, 128], F32R, tag="wu")
                    nc.sync.dma_start(wu_sb[:], wu_d[it])
                for c0, cs in CHUNKS:
                    pg = psA.tile([128, cs], F32, tag="pg")
                    pu = psA.tile([128, cs], F32, tag="pu")
                    for hc in range(HC):
                        nc.tensor.matmul(
                            pg[:],
                            wg_sb[:, hc, :],
                            x_sb[:, hc, c0:c0 + cs],
                            start=(hc == 0), stop=(hc == HC - 1),
                        )
                    for hc in range(HC):
                        nc.tensor.matmul(
                            pu[:],
                            wu_sb[:, hc, :],
                            x_sb[:, hc, c0:c0 + cs],
                            start=(hc == 0), stop=(hc == HC - 1),
                        )
                    silu_sb = spool.tile([128, cs], F32, tag="silu")
                    if use_silu:
                        nc.scalar.activation(
                            silu_sb[:], pg[:], mybir.ActivationFunctionType.Silu
                        )
                    else:
                        # CoreSim lacks Silu; decompose as g*sigmoid(g)
                        sig_sb = spool.tile([128, cs], F32, tag="sig")
                        nc.scalar.activation(
                            sig_sb[:], pg[:], mybir.ActivationFunctionType.Sigmoid
                        )
                        nc.vector.tensor_mul(silu_sb[:], sig_sb[:], pg[:])
                    nc.vector.tensor_mul(
                        inter_sb[:, it, c0:c0 + cs], silu_sb[:], pu[:]
                    )

            # Stage B: yT[h, c] = sum_i w_down[h, i] * interT[i, c]
            for ht in range(HT):
                wd_sb = wpool.tile([128, IT, 128], F32R, tag="wd")
                nc.sync.dma_start(wd_sb[:], wd_d[ht])
                y_sb = ypool.tile([128, C_PAD], F32, tag="y")
                for c0, cs in CHUNKS:
                    py = psB.tile([128, cs], F32, tag="py")
                    for ic in range(IT):
                        nc.tensor.matmul(
                            py[:],
                            wd_sb[:, ic, :],
                            inter_sb[:, ic, c0:c0 + cs],
                            start=(ic == 0), stop=(ic == IT - 1),
                        )
                    nc.vector.tensor_copy(y_sb[:, c0:c0 + cs], py[:])
                    # DMA each chunk out as soon as it is copied (shrinks the
                    # kernel tail vs one per-ht DMA after the last chunk)
                    nc.sync.dma_start(y_d[ht, :, c0:c0 + cs],
                                      y_sb[:, c0:c0 + cs])

    nc.compile()
    return nc


def _get_program():
    global _compiled_nc
    if _compiled_nc is None:
        _compiled_nc = _build_program()
    return _compiled_nc


def _router(x, gate_w):
    """Host router in float64.  Returns per-token top-2 selection, combine
    weights, and the aux outputs (all matching the reference numerics)."""
    logits = x.astype(np.float64) @ gate_w.T.astype(np.float64)  # [T, E]
    logits -= logits.max(axis=-1, keepdims=True)
    p = np.exp(logits)
    p /= p.sum(axis=-1, keepdims=True)
    order = np.argsort(-p, axis=-1, kind="stable")
    sel = order[:, :TOP_K]                       # [T, K]
    w = np.take_along_axis(p, sel, axis=-1)      # [T, K]
    w = w / w.sum(axis=-1, keepdims=True)
    counts = np.bincount(sel.ravel(), minlength=E).astype(np.float32)
    expert_probs = counts / np.float32(T)
    aux_loss = np.float32(
        AUX_COEF * np.sum((expert_probs.astype(np.float64) - 1.0 / E) ** 2)
    )
    z_loss = np.float32(Z_COEF * np.sum(w.astype(np.float32) ** 2))
    return sel, w.astype(np.float32), counts, expert_probs, aux_loss, z_loss


def _expert_host(x_e, wg, wu, wd):
    """Host fallback for capacity-overflow tokens (f32 BLAS; ~1e-6 relative
    error -- far below the device path's fp32r ~2e-4)."""
    g = x_e @ wg.T
    u = x_e @ wu.T
    inter = (g / (1.0 + np.exp(-g.astype(np.float64)))).astype(np.float32) * u
    return inter @ wd.T


def kernel(hidden_states, gate_w, w_gate, w_up, w_down):
    from concourse.bass_utils import run_bass_kernel_spmd

    x = np.ascontiguousarray(np.asarray(hidden_states, np.float32).reshape(T, H))
    gate_w = np.asarray(gate_w, np.float32)
    w_gate = np.asarray(w_gate, np.float32)
    w_up = np.asarray(w_up, np.float32)
    w_down = np.asarray(w_down, np.float32)

    sel, w, counts, expert_probs, aux_loss, z_loss = _router(x, gate_w)

    # Dispatch: token lists per expert
    tok_idx = [np.where((sel == e).any(axis=1))[0] for e in range(E)]
    tok_w = [
        w[tok_idx[e], (sel[tok_idx[e]] == e).argmax(axis=1)] for e in range(E)
    ]

    in_maps = []
    overflow = []  # (e, token_ids) handled on host
    for e in range(E):
        idx = tok_idx[e]
        if len(idx) > C_PAD:
            overflow.append((e, idx[C_PAD:]))
            idx = idx[:C_PAD]
        xs = x[idx]  # [cnt, H]
        xp = np.zeros((128, HC, C_PAD), np.float32)
        # [p, hc, c] = x[c, hc*128+p]
        xp[:, :, : len(idx)] = xs.T.reshape(HC, 128, len(idx)).transpose(1, 0, 2)
        wg_p = np.ascontiguousarray(
            w_gate[e].reshape(IT, 128, HC, 128).transpose(0, 3, 2, 1)
        )
        wu_p = np.ascontiguousarray(
            w_up[e].reshape(IT, 128, HC, 128).transpose(0, 3, 2, 1)
        )
        wd_p = np.ascontiguousarray(
            w_down[e].reshape(HT, 128, IT, 128).transpose(0, 3, 2, 1)
        )
        in_maps.append({"xT": xp, "wg": wg_p, "wu": wu_p, "wd": wd_p})

    nc = _get_program()
    res = run_bass_kernel_spmd(nc, in_maps, list(range(N_CORES)))

    out = np.zeros((T, H), np.float32)
    for e in range(E):
        idx = tok_idx[e]
        n = min(len(idx), C_PAD)
        y = res.results[e]["yT"].reshape(H, C_PAD)[:, :n]  # [H, n]
        out[idx[:n]] += tok_w[e][:n, None] * y.T
    for e, idx in overflow:
        we = tok_w[e][C_PAD:]
        y = _expert_host(x[idx], w_gate[e], w_up[e], w_down[e])
        out[idx] += we[:, None] * y

    out = out.reshape(B, S, H)
    return out, aux_loss, z_loss, counts, expert_probs
